# revision 13
# baseline (speedup 1.0000x reference)
"""Trainium2 Bass kernel for nn_DiffusionBlock (anisotropic diffusion step).

Sharding: pure data-parallel over batch. 16 batches -> 8 cores x 2 batches;
each core processes 4 images (2 batches x 2 channels) of 768x768.

Math (validated vs reference to 9e-8 rel):
  grid 769x769 (i,j in 0..768), pu = edge-padded u (clamp at row/col 767)
  P00=pu[i,j] P01=pu[i,j+1] P10=pu[i+1,j] P11=pu[i+1,j+1]
  g1 = P11-P00 ; g2 = P01-P10 ; m = P01+P10-P00-P11
  k4 = tau/8 ; k4m = tau*(1-2*alpha)/8
  s12 = k4*((a+b)*g1 + (a-b)*g2)   d12 = k4m*(a-|b|)*m
  s34 = k4*((b+c)*g1 + (b-c)*g2)   d34 = k4m*(c-|b|)*m    (a,b,c cropped [1:,1:])
  X = dcol(s12) ; Y = dcol(d12) ; S = s34[j+1]+s34[j] ; T = d34[j+1]-d34[j]
  Z = Y+S+T ; P = X+Z ; Q = X-Z
  out = u + P[i+1,j] + Q[i,j]

Row tiling: 128-partition tiles over grid rows [t0, t0+127]; out rows
[t0, t0+126] per tile. Tiles t0=0,127,...,635 need no row clamping at all.
The last tile runs at t0=640 (all loads direct) with 3 tiny patch ops for
grid row 767 semantics (E=0 there; g1=g2=coldiff(u[767])). Output row 767
(which needs grid-row-768 values) is produced by a separate small tail pass
using the identities at the bottom edge:
  s12[767]=2*k4*a'[767]*D767, s12[768]=2*k4*a'[768]*D767 (same for s34 w/ b'),
  d12=d34=0 at rows 767,768, where D767=coldiff(u[767]).
  out[767] = u[767] + P[768] + Q[767]
           = u[767] + dcol(s12[768]+s12[767]) + pair(s34[768]-s34[767])
  with pair(x)[j] = x[j+1]+x[j].
"""

import numpy as np
from contextlib import ExitStack

import concourse.bass as bass
import concourse.mybir as mybir
import concourse.tile as tile
from concourse.bacc import Bacc
from concourse.bass_utils import run_bass_kernel_spmd

F32 = mybir.dt.float32
OP = mybir.AluOpType

B, C, H, W = 16, 2, 768, 768
NCORES = 8
NIMG = 4          # images per core
IMGG = 2          # images per tile-group
GW = 770          # padded width for shifted reads
T0S = [0, 127, 254, 381, 508, 635, 640]


def _build(k4, k4m):
    nc = Bacc()
    u_d = nc.declare_dram_parameter("u", [NIMG, H, W], F32, isOutput=False)
    a_d = nc.declare_dram_parameter("a", [NIMG, H + 2, W + 2], F32, isOutput=False)
    b_d = nc.declare_dram_parameter("b", [NIMG, H + 2, W + 2], F32, isOutput=False)
    c_d = nc.declare_dram_parameter("c", [NIMG, H + 2, W + 2], F32, isOutput=False)
    o_d = nc.declare_dram_parameter("out", [NIMG, H, W], F32, isOutput=True)

    with tile.TileContext(nc) as tc, ExitStack() as ctx:
        loads = ctx.enter_context(tc.tile_pool(name="loads", bufs=2))
        scr = ctx.enter_context(tc.tile_pool(name="scr", bufs=1))
        outp = ctx.enter_context(tc.tile_pool(name="outp", bufs=2))

        def S(tag, w=GW):
            return scr.tile([128, IMGG, w], F32, tag=tag, name=tag)

        for t0 in T0S:
            last = t0 == 640
            for g in range(NIMG // IMGG):
                ig0 = g * IMGG
                # ---- loads: exactly one DMA descriptor per tile ----
                PU = loads.tile([128, IMGG, GW], F32, tag="pu")
                PU2 = loads.tile([128, IMGG, GW], F32, tag="pu2")
                nd2 = min(128, H - (t0 + 1))  # 128 except last tile (127)
                src = u_d[ig0:ig0 + IMGG, t0:t0 + 128, :]
                nc.sync.dma_start(out=PU[:, :, 0:W], in_=src.rearrange("i r c -> r i c"))
                src2 = u_d[ig0:ig0 + IMGG, t0 + 1:t0 + 1 + nd2, :]
                nc.sync.dma_start(out=PU2[0:nd2, :, 0:W], in_=src2.rearrange("i r c -> r i c"))
                if nd2 < 128:
                    # row clamp: grid row 768 = u row 767 (last tile only)
                    srcc = u_d[ig0:ig0 + IMGG, H - 1:H, :]
                    nc.sync.dma_start(out=PU2[nd2:128, :, 0:W], in_=srcc.rearrange("i r c -> r i c"))
                # clamp cols 768,769 <- col 767 (ScalarE broadcast copy)
                nc.scalar.copy(PU[:, :, W:W + 2], PU[:, :, W - 1:W].to_broadcast([128, IMGG, 2]))
                nc.scalar.copy(PU2[:, :, W:W + 2], PU2[:, :, W - 1:W].to_broadcast([128, IMGG, 2]))
                A = loads.tile([128, IMGG, 769], F32, tag="a")
                Bt = loads.tile([128, IMGG, 769], F32, tag="b")
                Ct = loads.tile([128, IMGG, 769], F32, tag="c")
                for dram, buf in ((a_d, A), (b_d, Bt), (c_d, Ct)):
                    srcw = dram[ig0:ig0 + IMGG, 1 + t0:1 + t0 + 128, 1:W + 2]
                    nc.sync.dma_start(out=buf[:], in_=srcw.rearrange("i r c -> r i c"))

                V = nc.vector
                GP = nc.gpsimd
                # Scratch slots s0..s11 are manually reused across arrays with
                # disjoint lifetimes (SBUF budget); Tile serializes WAR.
                # ---- stage A: stencil differences (769 wide) ----
                E = S("s0", GW)
                V.tensor_sub(E[:], PU2[:], PU[:])
                g1 = S("s1", 769)
                V.tensor_sub(g1[:], PU2[:, :, 1:GW], PU[:, :, 0:769])
                g2 = S("s2", 769)
                V.tensor_sub(g2[:], PU[:, :, 1:GW], PU2[:, :, 0:769])
                m = S("s3", 769)
                V.tensor_sub(m[:], E[:, :, 0:769], E[:, :, 1:GW])

                # ---- weight combos (769 wide) on gpsimd ----
                ab_p = S("s0", 769)
                GP.tensor_add(ab_p[:], A[:], Bt[:])
                ab_m = S("s4", 769)
                GP.tensor_sub(ab_m[:], A[:], Bt[:])
                bc_p = S("s5", 769)
                GP.tensor_add(bc_p[:], Bt[:], Ct[:])
                bc_m = S("s6", 769)
                GP.tensor_sub(bc_m[:], Bt[:], Ct[:])
                absB = S("s7", 769)
                V.scalar_tensor_tensor(absB[:], Bt[:], -1.0, Bt[:], OP.mult, OP.max)
                am = S("s10", 769)
                GP.tensor_sub(am[:], A[:], absB[:])
                cm = S("s11", 769)
                GP.tensor_sub(cm[:], Ct[:], absB[:])

                # ---- products (769 wide) ----
                t1 = S("s7", 769)
                V.scalar_tensor_tensor(t1[:], ab_p[:], k4, g1[:], OP.mult, OP.mult)
                t2 = S("s9", 769)
                V.scalar_tensor_tensor(t2[:], ab_m[:], k4, g2[:], OP.mult, OP.mult)
                s12 = S("s0", 769)
                V.tensor_add(s12[:], t1[:], t2[:])
                t3 = S("s7", 769)
                V.scalar_tensor_tensor(t3[:], bc_p[:], k4, g1[:], OP.mult, OP.mult)
                t4 = S("s9", 769)
                V.scalar_tensor_tensor(t4[:], bc_m[:], k4, g2[:], OP.mult, OP.mult)
                s34 = S("s4", 769)
                V.tensor_add(s34[:], t3[:], t4[:])
                d12 = S("s1", 769)
                V.scalar_tensor_tensor(d12[:], am[:], k4m, m[:], OP.mult, OP.mult)
                d34 = S("s2", 769)
                V.scalar_tensor_tensor(d34[:], cm[:], k4m, m[:], OP.mult, OP.mult)

                # ---- column-difference stage (768 wide) ----
                X = S("s3", W)
                V.tensor_sub(X[:], s12[:, :, 1:769], s12[:, :, 0:W])
                Y = S("s5", W)
                V.tensor_sub(Y[:], d12[:, :, 1:769], d12[:, :, 0:W])
                Sm = S("s0", W)
                V.tensor_add(Sm[:], s34[:, :, 1:769], s34[:, :, 0:W])
                T = S("s6", W)
                V.tensor_sub(T[:], d34[:, :, 1:769], d34[:, :, 0:W])
                R = S("s4", W)
                V.tensor_add(R[:], Sm[:], T[:])
                Z = S("s7", W)
                V.tensor_add(Z[:], Y[:], R[:])
                P = S("s1", W)
                V.tensor_add(P[:], X[:], Z[:])
                Q = S("s2", W)
                V.tensor_sub(Q[:], X[:], Z[:])

                # P shifted down one row (partition shift via SBUF->SBUF DMA)
                Pd = outp.tile([128, IMGG, W], F32, tag="pd")
                nc.gpsimd.dma_start(out=Pd[0:127, :, :], in_=P[1:128, :, :])

                Wt = S("s0", W)
                V.tensor_add(Wt[0:127], Pd[0:127], Q[0:127])
                ot = outp.tile([128, IMGG, W], F32, tag="ot")
                V.tensor_add(ot[0:127], Wt[0:127], PU[0:127, :, 0:W])

                # store out rows [t0, t0+126] (last tile: only rows 762..766)
                if not last:
                    p0, p1, r0 = 0, 127, t0
                else:
                    p0, p1, r0 = 122, 127, 762
                dst = o_d[ig0:ig0 + IMGG, r0:r0 + (p1 - p0), :]
                nc.gpsimd.dma_start(out=dst.rearrange("i r c -> r i c"), in_=ot[p0:p1, :, 0:W])

        # ---- tail pass: output row 767, all 4 images on partitions 0..3 ----
        tp = ctx.enter_context(tc.tile_pool(name="tail", bufs=1))
        U7 = tp.tile([4, GW], F32, name="u7")
        nc.sync.dma_start(out=U7[:, 0:W], in_=u_d[:, H - 1, :])
        nc.scalar.copy(U7[:, W:W + 2], U7[:, W - 1:W].to_broadcast([4, 2]))
        A7 = tp.tile([4, 2, 769], F32, name="a7")   # a' rows 767,768
        B7 = tp.tile([4, 2, 769], F32, name="b7")
        nc.sync.dma_start(out=A7[:], in_=a_d[:, H:H + 2, 1:W + 2])
        nc.sync.dma_start(out=B7[:], in_=b_d[:, H:H + 2, 1:W + 2])
        V = nc.vector
        D7 = tp.tile([4, 769], F32, name="d7")
        V.tensor_sub(D7[:], U7[:, 1:GW], U7[:, 0:769])
        aa = tp.tile([4, 769], F32, name="aa")   # a'[767] + a'[768]
        V.tensor_add(aa[:], A7[:, 0, :], A7[:, 1, :])
        bb = tp.tile([4, 769], F32, name="bb")   # b'[768] - b'[767]
        V.tensor_sub(bb[:], B7[:, 1, :], B7[:, 0, :])
        sA = tp.tile([4, 769], F32, name="sa")   # s12[768]+s12[767] = 2*k4*aa*D7
        V.scalar_tensor_tensor(sA[:], aa[:], 2.0 * k4, D7[:], OP.mult, OP.mult)
        sB = tp.tile([4, 769], F32, name="sb")   # s34[768]-s34[767] = 2*k4*bb*D7
        V.scalar_tensor_tensor(sB[:], bb[:], 2.0 * k4, D7[:], OP.mult, OP.mult)
        tX = tp.tile([4, W], F32, name="tx")
        V.tensor_sub(tX[:], sA[:, 1:769], sA[:, 0:W])
        tS = tp.tile([4, W], F32, name="ts")
        V.tensor_add(tS[:], sB[:, 1:769], sB[:, 0:W])
        tZ = tp.tile([4, W], F32, name="tz")
        V.tensor_add(tZ[:], tX[:], tS[:])
        o7 = tp.tile([4, W], F32, name="o7")
        V.tensor_add(o7[:], tZ[:], U7[:, 0:W])
        nc.gpsimd.dma_start(out=o_d[:, H - 1, :], in_=o7[:])
    nc.finalize()
    return nc


_cache = {}


def _get_nc(k4, k4m):
    key = (k4, k4m)
    if key not in _cache:
        _cache[key] = _build(k4, k4m)
    return _cache[key]


def kernel(u, a, b, c, grad_x1, grad_x2, grad_y1, grad_y2, alpha, tau):
    u = np.ascontiguousarray(np.asarray(u, dtype=np.float32))
    a = np.ascontiguousarray(np.asarray(a, dtype=np.float32))
    b = np.ascontiguousarray(np.asarray(b, dtype=np.float32))
    c = np.ascontiguousarray(np.asarray(c, dtype=np.float32))
    alpha_f = float(np.asarray(alpha))
    tau_f = float(np.asarray(tau))
    k4 = tau_f / 8.0
    k4m = tau_f * (1.0 - 2.0 * alpha_f) / 8.0

    nc = _get_nc(k4, k4m)

    bpc = B // NCORES  # batches per core
    in_maps = []
    for k in range(NCORES):
        sl = slice(bpc * k, bpc * (k + 1))
        in_maps.append({
            "u": np.ascontiguousarray(u[sl].reshape(NIMG, H, W)),
            "a": np.ascontiguousarray(a[sl].reshape(NIMG, H + 2, W + 2)),
            "b": np.ascontiguousarray(b[sl].reshape(NIMG, H + 2, W + 2)),
            "c": np.ascontiguousarray(c[sl].reshape(NIMG, H + 2, W + 2)),
        })

    res = run_bass_kernel_spmd(nc, in_maps, list(range(NCORES)))
    out = np.empty((B, C, H, W), dtype=np.float32)
    for k in range(NCORES):
        out[bpc * k:bpc * (k + 1)] = res.results[k]["out"].reshape(bpc, C, H, W)
    return out
